# revision 3
# baseline (speedup 1.0000x reference)
"""
Trainium2 Bass kernel for nn_BaseDecoder (9x9 local cost volume / spatial
correlation, kernel_size=1):

    out[b, di*9+dj, y, x] = sum_c t1[b,c,y,x] * t2p[b,c,y+di,x+dj]

t1/t2: [4, 128, 128, 256] f32, out: [4, 81, 128, 256] f32, zero-padded t2.

Strategy (v6)
-------------
8 cores = (batch 4) x (H halves 2), fully data parallel; each core gets its
t1 shard [128c, 64y, 256x] and a zero-padded t2 slab [128c, 72y, 264x]
(4-row/4-col halo baked in on host), both pre-cast to bf16 on host (halves
HBM reads; rel-err budget 2e-2 >> bf16's ~4e-3).

Channels live on partitions, so the 81 shifted dot products come from the
PE.  Unlike v5 (3 banded f32r matmuls of N=408 per (y, x-block) whose 9
useful diagonals then needed a GPSIMD ap_gather + DVE/ACT quadrant
extracts), v6 issues FOUR quadrant matmuls per (y, x-block): quadrant q
uses lhsT = t1[c, 32 cols at 128*xb+32q] (M=32) and rhs = t2slab[c, y:y+9,
40-wide window at 128*xb+32q] (N=9*40=360, one PSUM bank).  PSUM thus
directly holds the 32-aligned windows ps[x, di, j] = corr(x, di, dj=j-x%32)
-- the whole deskew collapses to ONE 128-wide PSUM->SBUF bf16 copy per
(y, x-block) (360 elems vs 1224), alternating DVE (xb=0) / ACT (xb=1).
No GPSIMD at all.  PE streams 4*360 = 1440 cols/(y,xb) vs v5's 1224 (+18%),
at 1 col/cyc (bf16).  Outputs (40-wide windows, bf16) DMA out in YB-row
batches; the residual within-32 skew j = (x%32)+dj is a host-side
take_along_axis, and host upcasts bf16 -> f32.

Per-core/sweep budget (cost model): PE ~80us (wall), DVE/ACT ~20us each,
HBM 9.0MB in / 11.8MB out -> ~25/~33us on separate queues.  v5 sim:
241us; v6 sim: ~100us.
"""

import os
import sys

sys.path.insert(0, "/opt/trn_rl_repo")

from contextlib import ExitStack

import numpy as np
import ml_dtypes

import concourse.bacc as bacc
import concourse.bass as bass
import concourse.mybir as mybir
import concourse.tile as tile
from concourse.bass_utils import run_bass_kernel_spmd

MD = 4
D = 9  # patch size (9x9 displacements)
B, C, H, W = 4, 128, 128, 256
HSH = H // 2  # 64 rows per shard
T2R = HSH + 2 * MD  # 72 t2 slab rows
T2C = W + 2 * MD  # 264 t2 slab cols
QW = 40  # 32-aligned window width per di (32 + 8)
NQ = 4  # quadrants per 128-x block
SLOT = D * QW  # 360 psum cols per (y, xb)
YB = int(os.environ.get("KERNEL_YB", "8"))  # y rows per output DMA batch

F32 = mybir.dt.float32
BF16 = mybir.dt.bfloat16

# internal whole-kernel repeat count (for HW timing via differencing)
REPEAT = int(os.environ.get("KERNEL_REPEAT", "1"))
# comma list of stages to drop, for ablation: mm,evac,outdma,indma
ABLATE = set(filter(None, os.environ.get("KERNEL_ABLATE", "").split(",")))
# engine for the PSUM->SBUF evac: split = DVE(xb0)+ACT(xb1)
EVAC = os.environ.get("KERNEL_EVAC", "split")
IN_CHUNKS = int(os.environ.get("KERNEL_IN_CHUNKS", "8"))
PSUM_BUFS = int(os.environ.get("KERNEL_PSUM_BUFS", "4"))


def build_program():
    nc = bacc.Bacc("TRN2")

    t1s = nc.declare_dram_parameter("t1s", [C, HSH, W], BF16, isOutput=False)
    t2s = nc.declare_dram_parameter("t2s", [C, T2R, T2C], BF16, isOutput=False)
    out40 = nc.declare_dram_parameter(
        "out40", [HSH // YB, C, YB * 2 * SLOT], BF16, isOutput=True
    )

    do_mm = "mm" not in ABLATE
    do_evac = do_mm and "evac" not in ABLATE
    do_outdma = do_evac and "outdma" not in ABLATE

    with ExitStack() as ctx:
        tc = ctx.enter_context(tile.TileContext(nc))
        inp = ctx.enter_context(tc.tile_pool(name="inp", bufs=1))
        psump = ctx.enter_context(tc.tile_pool(name="psum", bufs=PSUM_BUFS, space="PSUM"))
        stgp = ctx.enter_context(tc.tile_pool(name="stg", bufs=2))

        t1sb = inp.tile([C, HSH, W], BF16)
        t2sb = inp.tile([C, T2R, T2C], BF16)

        rep_ctx = tc.For_i(0, REPEAT, 1) if REPEAT > 1 else None
        if rep_ctx is not None:
            ctx.enter_context(rep_ctx)

        # input DMAs, chunked so compute can start before the full slab lands
        for ch in range(IN_CHUNKS) if "indma" not in ABLATE else []:
            r0, r1 = HSH * ch // IN_CHUNKS, HSH * (ch + 1) // IN_CHUNKS
            nc.sync.dma_start(t1sb[:, r0:r1, :], t1s[:, r0:r1, :])
            s0, s1 = T2R * ch // IN_CHUNKS, T2R * (ch + 1) // IN_CHUNKS
            nc.sync.dma_start(t2sb[:, s0:s1, :], t2s[:, s0:s1, :])

        for yb in range(HSH // YB):
            stg = (
                stgp.tile([C, YB, 2, SLOT], BF16, name="stg") if do_evac else None
            )
            for y8 in range(YB):
                y = yb * YB + y8
                for xb in range(2):
                    if not do_mm:
                        continue
                    ps = psump.tile([C, SLOT], F32, name="ps")
                    for q in range(NQ):
                        x0 = 128 * xb + 32 * q
                        nc.tensor.matmul(
                            ps[32 * q : 32 * q + 32, :],
                            t1sb[:, y, x0 : x0 + 32],
                            t2sb[:, y : y + D, x0 : x0 + QW],
                            start=True,
                            stop=True,
                            tile_position=(0, 32 * q),
                        )
                    if do_evac:
                        dst = stg[:, y8, xb, :]
                        if EVAC == "dve" or (EVAC == "split" and xb == 0):
                            nc.vector.tensor_copy(dst, ps[:])
                        else:
                            nc.scalar.copy(dst, ps[:])
            if do_outdma:
                nc.scalar.dma_start(out40[yb], stg.rearrange("p a b c -> p (a b c)"))

    nc.finalize()
    return nc


_PROG_CACHE = {}


def get_program():
    key = (REPEAT, YB, EVAC, IN_CHUNKS, PSUM_BUFS, tuple(sorted(ABLATE)))
    if key not in _PROG_CACHE:
        _PROG_CACHE[key] = build_program()
    return _PROG_CACHE[key]


def make_in_maps(t1: np.ndarray, t2: np.ndarray):
    t1 = np.asarray(t1, dtype=np.float32)
    t2 = np.asarray(t2, dtype=np.float32)
    t2p = np.zeros((B, C, H + 2 * MD, W + 2 * MD), dtype=ml_dtypes.bfloat16)
    t2p[:, :, MD : MD + H, MD : MD + W] = t2
    t1b = t1.astype(ml_dtypes.bfloat16)
    in_maps = []
    for core in range(8):
        b, h2 = divmod(core, 2)
        y0 = HSH * h2
        in_maps.append(
            {
                "t1s": np.ascontiguousarray(t1b[b, :, y0 : y0 + HSH, :]),
                "t2s": np.ascontiguousarray(t2p[b, :, y0 : y0 + T2R, :]),
            }
        )
    return in_maps


# host-side residual deskew index: I40[xl, di, dj] = di*40 + (xl%32) + dj
_XL = np.arange(128)
_I40 = (
    np.arange(D)[None, :, None] * QW
    + (_XL % 32)[:, None, None]
    + np.arange(D)[None, None, :]
)  # [128, 9, 9]


def assemble_out(results) -> np.ndarray:
    out = np.empty((B, D * D, H, W), dtype=np.float32)
    idx = np.broadcast_to(
        _I40.reshape(1, 1, 1, 128, D * D), (HSH // YB, YB, 2, 128, D * D)
    )
    for core in range(8):
        b, h2 = divmod(core, 2)
        y0 = HSH * h2
        o = results[core]["out40"].reshape(HSH // YB, C, YB, 2, SLOT)
        o = o.transpose(0, 2, 3, 1, 4)  # [yb, y8, xb, xl, w]
        g = np.take_along_axis(o.astype(np.float32), idx, axis=4)
        g = g.transpose(4, 0, 1, 2, 3)  # [81, yb, y8, xb, xl]
        out[b, :, y0 : y0 + HSH, :] = g.reshape(D * D, HSH, W)
    return out


def run(t1: np.ndarray, t2: np.ndarray, trace: bool = False, **kw):
    nc = get_program()
    in_maps = make_in_maps(t1, t2)
    res = run_bass_kernel_spmd(nc, in_maps, list(range(8)), trace=trace, **kw)
    return assemble_out(res.results), res


def kernel(t1: np.ndarray, t2: np.ndarray) -> np.ndarray:
    return run(t1, t2)[0]


if __name__ == "__main__":
    t1 = np.random.randn(B, C, H, W).astype(np.float32)
    t2 = np.random.randn(B, C, H, W).astype(np.float32)
    out = kernel(t1, t2)
    print(out.shape, out.dtype)
